# revision 18
# baseline (speedup 1.0000x reference)
"""Trainium2 Bass kernel for KeyValueAttention (4-head masked attention, gated combine).

v3 strategy (8 NeuronCores, query-dim sharded, 512 queries/core):
  Transposed space throughout (keys/features on partitions, queries on free dim).
  - All projections (Q/K/V) are fp8e4 DoubleRow matmuls (contraction 256 as
    2x128 k-tiles) -> 0.5 cycles/row on the PE.
  - TWO PASSES over the keys, one per head pair. Per pass the scores psum
    rotates through 3 buffers (6 banks) and the 2 AV accumulators use 2 banks,
    fitting the 8-bank PSUM while keeping the exp pipeline deep.
  - Scores: fp8 DR matmul, contraction A=64 as 2x32 k-tiles:
    lhsT = K^T chunk [32, 2, 128], rhs = Q^T [32, 2, 512] -> psum [128k, 512q].
  - Masked exp alternates engines by chunk parity:
      * even chunks (ACT): mask pre-added as -160 bias via an identity DR
        matmul opening the psum accumulation group, then ACT Exp (scale=1/8).
      * odd chunks (DVE): custom DVE op computes cubic-poly exp(s/8) * mask
        stream in one pass (Src0 = psum scores, Src1 = fp8 mask from SBUF).
    Both write em directly as fp8e4.
  - The fp8 mask image for all chunks is DMA'd into SBUF once (pass 1) and
    reused from SBUF in pass 2.
  - AV: fp8 DR over chunk pairs: lhsT = Vaug [128, 2, 65], rhs = em
    [128, 2, 512] -> psum [65, 512] per head; row 64 = softmax denominator.
  - The pass-2 K/V build matmuls are interleaved into the pass-1 chunk loop.

Host side only reshapes/slices/transposes/casts inputs (no reference math).
"""

import os
import numpy as np

NQ, NK, DC, A, H, DO = 4096, 8192, 256, 64, 4, 256
NCORES = 8
NQC = NQ // NCORES   # 512 queries per core
KC = 128             # keys per chunk
NKC = NK // KC       # 64 chunks
NPAIR = NKC // 2     # 32 chunk pairs

# chunk -> exp/mask path:
#   'C'  = DVE custom op (poly exp * mask stream), one pass
#   'Bd' = ACT exp (unmasked) + DVE in-place mask multiply
#   'Bp' = ACT exp (unmasked) + gpsimd in-place mask multiply
def _chunk_type(c):
    if c % 4 == 1:
        return "C"
    if c % 16 in (3, 7, 11):
        return "Bp"
    return "Bd"

CHUNK_TYPE = [_chunk_type(c) for c in range(NKC)]

_cache = {}


# ---------------------------------------------------------------------------
# exp polynomial fit (shared host/device constants)
# ---------------------------------------------------------------------------
def _fit_exp_poly(scale=0.125, lo=-0.85, hi=0.85):
    """p(x) = 1 + b1 x + b2 x^2 + b3 x^3 ~ exp(x*scale) for x*scale in [lo,hi],
    relative-error weighted, p(0)=1 pinned."""
    t = np.linspace(lo, hi, 40001)
    w = 1.0 / np.exp(t)
    Amat = np.stack([t, t * t, t ** 3], axis=1) * w[:, None]
    a = np.linalg.lstsq(Amat, (np.exp(t) - 1.0) * w, rcond=None)[0]
    return [float(a[0] * scale), float(a[1] * scale ** 2), float(a[2] * scale ** 3)]


POLY_B = _fit_exp_poly()


def _register_dve_exp_op():
    """Define + register the custom DVE op (idempotent)."""
    from concourse.dve_spec import Spec, Src0, Src1, C0, C1, C2, One, lower
    from concourse.dve_ops import (
        DveOp, OPS, CUSTOM_DVE_SPECS, _SUB_OPCODE_FOR_NAME, _CUSTOM_DVE_ROW_BASE,
    )
    from concourse.dve_table_gen import dve_ver_for
    from concourse.dve_uop import DveOpSpec

    name = "EXP_POLY_MASK_ANT"
    if name in _SUB_OPCODE_FOR_NAME:
        return next(op for op in OPS if op.name == name)

    body = (((Src0 * C2 + C1) * Src0 + C0) * Src0 + One) * Src1
    spec = Spec(
        body=body,
        reference=lambda in0, in1, s0, s1, imm2: (
            (((in0 * imm2 + s1) * in0 + s0) * in0 + 1.0) * in1
        ),
    )
    op = DveOp(name, spec, subdim=False, uops_sha={})
    ver = dve_ver_for("TRN2")
    op.uops_sha[ver] = DveOpSpec(
        name=name, opcode=31, uops=lower(spec, ver=ver), rd1_en=True
    ).sha(ver)
    OPS.append(op)
    CUSTOM_DVE_SPECS[name] = spec
    _SUB_OPCODE_FOR_NAME[name] = _CUSTOM_DVE_ROW_BASE + len(OPS) - 1
    return op


# ---------------------------------------------------------------------------
# kernel build
# ---------------------------------------------------------------------------
def _build_kernel():
    import concourse.bacc as bacc
    import concourse.mybir as mybir
    from concourse.tile import TileContext
    from concourse.masks import make_identity

    EXP_OP = _register_dve_exp_op()

    F32 = mybir.dt.float32
    BF16 = mybir.dt.bfloat16
    FP8 = mybir.dt.float8e4
    AF = mybir.ActivationFunctionType
    ALU = mybir.AluOpType
    DR = mybir.MatmulPerfMode.DoubleRow

    nc = bacc.Bacc(None, target_bir_lowering=False, debug=False)

    def eng_copy(eng, dst, src):
        # NOTE: gpsimd cannot access PSUM on HW; keep psum reads on scalar/vector.
        if eng is nc.scalar:
            nc.scalar.copy(dst, src)
        else:
            eng.tensor_copy(dst, src)

    # ---- DRAM inputs (per core) ----
    xqtb = nc.dram_tensor("xqtb", [128, 2, NQC], BF16, kind="ExternalInput")
    xkt8 = nc.dram_tensor("xkt8", [128, 2, NK], FP8, kind="ExternalInput")
    wqb = nc.dram_tensor("wqb", [128, 2, H, A], BF16, kind="ExternalInput")
    wkTb = nc.dram_tensor("wkTb", [64, 2, H, 128], BF16, kind="ExternalInput")
    wv8 = nc.dram_tensor("wv8", [128, 2, H * A], FP8, kind="ExternalInput")
    wgtb = nc.dram_tensor("wgtb", [128, 2, H], BF16, kind="ExternalInput")
    bg = nc.dram_tensor("bg", [H, 1], F32, kind="ExternalInput")
    wo = nc.dram_tensor("wo", [A, DO], F32, kind="ExternalInput")
    bo = nc.dram_tensor("bo", [1, DO], F32, kind="ExternalInput")
    maskx = nc.dram_tensor("maskx", [NKC, 128, 2 * NQC], FP8, kind="ExternalInput")
    out = nc.dram_tensor("out", [NQC, DO], F32, kind="ExternalOutput")

    with TileContext(nc) as tc:
        with tc.sbuf_pool(name="consts", bufs=1) as cpool:
            # ---- constants ----
            wq_t = cpool.tile([128, 2, H, A], BF16)
            nc.sync.dma_start(wq_t, wqb[:])
            wkT_t = cpool.tile([64, 2, H, 128], BF16)
            nc.sync.dma_start(wkT_t, wkTb[:])
            wv_t = cpool.tile([128, 2, H * A], FP8)
            nc.sync.dma_start(wv_t, wv8[:])
            wgt_t = cpool.tile([128, 2, H], BF16)
            nc.sync.dma_start(wgt_t, wgtb[:])
            bg_t = cpool.tile([H, 1], F32)
            nc.sync.dma_start(bg_t, bg[:])
            xqtb_t = cpool.tile([128, 2, NQC], BF16)
            nc.sync.dma_start(xqtb_t, xqtb[:])
            xkt_t = cpool.tile([128, 2, NK], FP8)
            nc.sync.dma_start(xkt_t, xkt8[:])
            bo_t = cpool.tile([1, DO], F32)
            nc.sync.dma_start(bo_t, bo[:])
            wo_t = cpool.tile([A, DO], F32)
            nc.sync.dma_start(wo_t, wo[:])
            woaug = cpool.tile([A + 1, DO + 1], BF16)
            nc.vector.memset(woaug, 0.0)
            nc.any.tensor_copy(woaug[:A, :DO], wo_t)
            nc.vector.memset(woaug[A : A + 1, DO : DO + 1], 1.0)
            ones1 = cpool.tile([1, 128], F32)
            nc.vector.memset(ones1, 1.0)
            identity = cpool.tile([128, 128], F32)
            make_identity(nc, identity)

            # ---- persistent operand tiles ----
            # QW[h] = Wk_h @ Q_h^T in fp8 DR layout [128, 2, NQC] (c = i*128+p)
            qw8 = [cpool.tile([128, 2, NQC], FP8, name=f"qw{h}") for h in range(H)]
            qt_bf = cpool.tile([64, H, NQC], BF16)
            # last dim padded to 80 so the AV DoubleRow k-tile step is %16==0
            vaug = cpool.tile([128, H, NKC, 80], FP8)
            # only the augmented ones-column needs initialization
            nc.gpsimd.memset(vaug[:, :, :, A : A + 1], 1.0)
            gates = cpool.tile([H, NQC], F32)
            # whole mask image, SBUF resident (written in pass 1, reused pass 2)
            mask_sb = cpool.tile([128, NKC, 2 * NQC], FP8)
            nh = [cpool.tile([A + 1, NQC], BF16, name=f"nh{h}") for h in range(H)]

            KBLK = 512

            with (
                tc.psum_pool(name="pmain", bufs=1) as pm,
                tc.sbuf_pool(name="ms", bufs=1) as ms,
            ):
                # ---- build helpers (all ride the "sset" psum rotation) ----
                def sset_tile():
                    s4 = pm.tile([128, 2, NQC], F32, tag="sset", bufs=3,
                                 name="s4")
                    return s4

                def build_qt(hpair):
                    # Q_h^T = Wq_h^T @ x_Q^T  (bf16), heads 2*hpair, 2*hpair+1
                    qps = sset_tile()
                    for hh in range(2):
                        h = 2 * hpair + hh
                        for i in range(2):
                            nc.tensor.matmul(
                                qps[0:64, hh, :], wq_t[:, i, h, :],
                                xqtb_t[:, i, :],
                                start=(i == 0), stop=(i == 1),
                            )
                        eng_copy((nc.scalar, nc.vector)[hh], qt_bf[:, h, :],
                                 qps[0:64, hh, :])

                def build_qw(h):
                    # QW_h = Wk_h @ Q_h^T -> fp8 [128, 2, NQC] (c = i*128+p)
                    qps = sset_tile()
                    for half in range(2):
                        nc.tensor.matmul(
                            qps[:, half, :], wkT_t[:, half, h, :],
                            qt_bf[:, h, :],
                            start=True, stop=True,
                        )
                        eng_copy((nc.scalar, nc.vector)[half],
                                 qw8[h][:, half, :], qps[:, half, :])

                def build_v(P, cp):  # one key chunk PAIR, heads of pair P
                    vps = sset_tile()
                    for s in range(2):
                        c = 2 * cp + s
                        nc.tensor.matmul(
                            vps[:, s, 0 : 2 * A],
                            xkt_t[:, :, c * KC : (c + 1) * KC],
                            wv_t[:, :, 2 * P * A : (2 * P + 2) * A],
                            start=True, stop=True, perf_mode=DR,
                        )
                        eng_copy(
                            (nc.scalar, nc.vector)[s],
                            vaug[:, 2 * P : 2 * P + 2, c, 0:A],
                            vps[:, s, 0 : 2 * A].rearrange("p (h a) -> p h a", h=2),
                        )

                # ---- upfront mask DMAs (8 batched, sync engine) ----
                for g in range(8):
                    nc.sync.dma_start(
                        mask_sb[:, 8 * g : 8 * (g + 1), :],
                        maskx[8 * g : 8 * (g + 1)].rearrange("c p q -> p c q"),
                    )

                # ---- build: Q, QW (all heads), gates, V for pass 0 ----
                for hpair in range(2):
                    build_qt(hpair)
                for h in range(H):
                    build_qw(h)
                g_ps = sset_tile()
                for i in range(2):
                    nc.tensor.matmul(
                        g_ps[0:4, 0, :], wgt_t[:, i, :], xqtb_t[:, i, :],
                        start=(i == 0), stop=(i == 1),
                    )
                nc.scalar.activation(gates, g_ps[0:4, 0, :], AF.Sigmoid,
                                     bias=bg_t[:], scale=1.0)

                for cp in range(NPAIR):
                    build_v(0, cp)

                # ---- two passes over keys, one head pair each ----
                for PASS in range(2):
                    h0 = 2 * PASS
                    avP = [
                        pm.tile([A + 1, NQC], F32, tag=f"av{hh}", bufs=1,
                                name=f"av{hh}")
                        for hh in range(2)
                    ]
                    for pair in range(NPAIR):
                        em_cur = ms.tile([128, 2, 2, NQC], FP8, tag="em", bufs=2)

                        if PASS == 0 and pair < NPAIR // 2:
                            # interleave pass-1 V build into the pass-0 loop
                            build_v(1, 2 * pair)
                            build_v(1, 2 * pair + 1)

                        s4s = []
                        for slot in range(2):
                            c = 2 * pair + slot
                            s4 = sset_tile()
                            s4s.append(s4)
                            for hh in range(2):
                                nc.tensor.matmul(
                                    s4[:, hh, :],
                                    xkt_t[:, :, c * KC : (c + 1) * KC],
                                    qw8[h0 + hh],
                                    start=True, stop=True, perf_mode=DR,
                                )
                        for slot in range(2):
                            c = 2 * pair + slot
                            ct = CHUNK_TYPE[c]
                            dst = em_cur[:, slot]
                            if ct == "C":
                                nc.vector._custom_dve(
                                    EXP_OP, out=dst, in0=s4s[slot],
                                    in1=mask_sb[:, c, :],
                                    s0=POLY_B[0], s1=POLY_B[1], imm2=POLY_B[2],
                                )
                            else:
                                nc.scalar.activation(
                                    dst, s4s[slot], AF.Exp, bias=0.0, scale=0.125
                                )
                                dflat = dst.rearrange("p h q -> p (h q)")
                                eng = nc.vector if ct == "Bd" else nc.gpsimd
                                eng.tensor_mul(dflat, dflat, mask_sb[:, c, :])
                        for hh in range(2):
                            nc.tensor.matmul(
                                avP[hh],
                                vaug[:, h0 + hh, 2 * pair : 2 * pair + 2, 0 : A + 1],
                                em_cur[:, :, hh, :],
                                start=(pair == 0), stop=(pair == NPAIR - 1),
                                perf_mode=DR,
                            )

                    for hh in range(2):
                        eng_copy((nc.scalar, nc.vector)[hh], nh[h0 + hh], avP[hh])

            # ---------------- epilogue ----------------
            with (
                tc.psum_pool(name="pe", bufs=1) as pm,
                tc.sbuf_pool(name="es", bufs=1) as ms,
            ):
                gt_ps = pm.tile([128, 4 * H], F32, tag="gt", bufs=1)
                for qtile in range(4):
                    nc.tensor.transpose(
                        gt_ps[:, qtile * H : qtile * H + H],
                        gates[:, qtile * 128 : (qtile + 1) * 128],
                        identity[:H, :H],
                    )
                gt_sb = ms.tile([128, 4 * H], F32, tag="gtsb", bufs=1)
                nc.any.tensor_copy(gt_sb, gt_ps)
                boB_ps = pm.tile([128, DO], F32, tag="bob", bufs=1)
                nc.tensor.matmul(boB_ps, ones1, bo_t, start=True, stop=True)
                boB = ms.tile([128, DO], F32, tag="bobsb", bufs=1)
                nc.any.tensor_copy(boB, boB_ps)
                for qtile in range(4):
                    acc = boB
                    for h in range(H):
                        p_ps = pm.tile([128, DO + 1], F32, tag="p", bufs=2)
                        nc.tensor.matmul(
                            p_ps,
                            nh[h][:, qtile * 128 : (qtile + 1) * 128],
                            woaug,
                            start=True, stop=True,
                        )
                        rden = ms.tile([128, 1], F32, tag="rden", bufs=2)
                        nc.vector.reciprocal(rden, p_ps[:, DO : DO + 1])
                        sc = ms.tile([128, 1], F32, tag="sc", bufs=2)
                        nc.any.tensor_mul(
                            sc, rden, gt_sb[:, qtile * H + h : qtile * H + h + 1]
                        )
                        nxt = ms.tile([128, DO], F32, tag=f"acc{h % 2}", bufs=2)
                        nc.vector.scalar_tensor_tensor(
                            nxt, p_ps[:, :DO], sc, acc,
                            op0=ALU.mult, op1=ALU.add,
                        )
                        acc = nxt
                    nc.sync.dma_start(
                        out[qtile * 128 : (qtile + 1) * 128, :], acc
                    )
    nc.finalize()
    return nc


# ---------------------------------------------------------------------------
# host-side input prep
# ---------------------------------------------------------------------------
def _to_f8(x):
    import ml_dtypes
    return np.ascontiguousarray(np.asarray(x, dtype=np.float32).astype(
        ml_dtypes.float8_e4m3fn))


def _to_bf16(x):
    import ml_dtypes
    return np.ascontiguousarray(np.asarray(x, dtype=np.float32).astype(
        ml_dtypes.bfloat16))


def _dr_c_layout(xT):
    """[C=256, N] -> [128, 2, N] with c = i*128 + p."""
    return np.ascontiguousarray(xT.reshape(2, 128, -1).transpose(1, 0, 2))


def _prep_shared(x_K, Wq, Wk, Wv, Wg, bg, Wo, bo):
    xkt = x_K.T  # [256, NK]
    xkt8 = _to_f8(_dr_c_layout(xkt))

    # wqb[p, i, h, a] = Wq[h, i*128+p, a]
    wqb = _to_bf16(Wq.transpose(1, 0, 2).reshape(2, 128, H, A).transpose(1, 0, 2, 3))
    # wkTb[a, half, h, m] = Wk[h, 128*half + m, a]
    wkTb = _to_bf16(
        Wk.reshape(H, 2, 128, A).transpose(3, 1, 0, 2)
    )
    arr = np.empty((128, 2, H * A), np.float32)
    for h in range(H):
        arr[:, :, h * A:(h + 1) * A] = Wv[h].reshape(2, 128, A).transpose(1, 0, 2)
    wv8 = _to_f8(arr)
    wgtb = _to_bf16(Wg.T.reshape(2, 128, H).transpose(1, 0, 2))
    return {
        "xkt8": xkt8, "wqb": wqb, "wkTb": wkTb, "wv8": wv8, "wgtb": wgtb,
        "bg": np.asarray(bg, np.float32).reshape(H, 1),
        "wo": np.ascontiguousarray(np.asarray(Wo, np.float32)),
        "bo": np.asarray(bo, np.float32).reshape(1, DO),
    }


def _prep_mask_core(mask_sl):
    """mask_sl: [NQC, NK] int32 -> maskx [NKC, 128, 2*NQC] fp8 (0/1, duplicated
    per head so [p, (h q)] reads align with the em layout)."""
    import ml_dtypes
    mt = mask_sl.T.astype(np.float32)  # [NK, NQC]
    m3 = mt.reshape(NKC, KC, NQC)
    maskx = np.concatenate([m3, m3], axis=2)  # [NKC, 128, 2*NQC]
    return np.ascontiguousarray(maskx.astype(ml_dtypes.float8_e4m3fn))


def kernel(x_Q, x_K, mask, Wq, Wk, Wv, Wg, bg, Wo, bo):
    from concourse.bass_utils import run_bass_kernel_spmd

    x_Q = np.asarray(x_Q, dtype=np.float32)
    x_K = np.asarray(x_K, dtype=np.float32)
    mask = np.asarray(mask, dtype=np.int32)

    shared = _prep_shared(
        x_K, np.asarray(Wq, np.float32), np.asarray(Wk, np.float32),
        np.asarray(Wv, np.float32), np.asarray(Wg, np.float32),
        bg, Wo, bo,
    )

    in_maps = []
    for cidx in range(NCORES):
        sl = slice(cidx * NQC, (cidx + 1) * NQC)
        xqt = x_Q[sl].T  # [256, NQC]
        m = {
            "xqtb": _to_bf16(_dr_c_layout(xqt)),
            "maskx": _prep_mask_core(mask[sl]),
        }
        m.update(shared)
        in_maps.append(m)

    if "nc" not in _cache:
        _cache["nc"] = _build_kernel()
    res = run_bass_kernel_spmd(
        _cache["nc"], in_maps, list(range(NCORES)),
        trace=bool(int(os.environ.get("BASS_KERNEL_TRACE", "0"))),
    )
    if res.exec_time_ns is not None:
        print(f"HW exec time: {res.exec_time_ns} ns")
    return np.concatenate([r["out"] for r in res.results], axis=0)


# revision 19
# speedup vs baseline: 1.0163x; 1.0163x over previous
"""Trainium2 Bass kernel for KeyValueAttention (4-head masked attention, gated combine).

v3 strategy (8 NeuronCores, query-dim sharded, 512 queries/core):
  Transposed space throughout (keys/features on partitions, queries on free dim).
  - All projections (Q/K/V) are fp8e4 DoubleRow matmuls (contraction 256 as
    2x128 k-tiles) -> 0.5 cycles/row on the PE.
  - TWO PASSES over the keys, one per head pair. Per pass the scores psum
    rotates through 3 buffers (6 banks) and the 2 AV accumulators use 2 banks,
    fitting the 8-bank PSUM while keeping the exp pipeline deep.
  - Scores: fp8 DR matmul, contraction A=64 as 2x32 k-tiles:
    lhsT = K^T chunk [32, 2, 128], rhs = Q^T [32, 2, 512] -> psum [128k, 512q].
  - Masked exp alternates engines by chunk parity:
      * even chunks (ACT): mask pre-added as -160 bias via an identity DR
        matmul opening the psum accumulation group, then ACT Exp (scale=1/8).
      * odd chunks (DVE): custom DVE op computes cubic-poly exp(s/8) * mask
        stream in one pass (Src0 = psum scores, Src1 = fp8 mask from SBUF).
    Both write em directly as fp8e4.
  - The fp8 mask image for all chunks is DMA'd into SBUF once (pass 1) and
    reused from SBUF in pass 2.
  - AV: fp8 DR over chunk pairs: lhsT = Vaug [128, 2, 65], rhs = em
    [128, 2, 512] -> psum [65, 512] per head; row 64 = softmax denominator.
  - The pass-2 K/V build matmuls are interleaved into the pass-1 chunk loop.

Host side only reshapes/slices/transposes/casts inputs (no reference math).
"""

import os
import numpy as np

NQ, NK, DC, A, H, DO = 4096, 8192, 256, 64, 4, 256
NCORES = 8
NQC = NQ // NCORES   # 512 queries per core
KC = 128             # keys per chunk
NKC = NK // KC       # 64 chunks
NPAIR = NKC // 2     # 32 chunk pairs

# chunk -> exp/mask path:
#   'C'  = DVE custom op (poly exp * mask stream), one pass
#   'Bd' = ACT exp (unmasked) + DVE in-place mask multiply
#   'Bp' = ACT exp (unmasked) + gpsimd in-place mask multiply
def _chunk_type(c):
    # 26 of 64 chunks pair ACT exp with a gpsimd mask-multiply; the rest run
    # the fused poly+mask on the DVE.
    return "Bp" if c % 5 < 2 else "C"

CHUNK_TYPE = [_chunk_type(c) for c in range(NKC)]

_cache = {}


# ---------------------------------------------------------------------------
# exp polynomial fit (shared host/device constants)
# ---------------------------------------------------------------------------
def _fit_exp_poly(scale=0.125, lo=-0.85, hi=0.85):
    """p(x) = 1 + b1 x + b2 x^2 + b3 x^3 ~ exp(x*scale) for x*scale in [lo,hi],
    relative-error weighted, p(0)=1 pinned."""
    t = np.linspace(lo, hi, 40001)
    w = 1.0 / np.exp(t)
    Amat = np.stack([t, t * t, t ** 3], axis=1) * w[:, None]
    a = np.linalg.lstsq(Amat, (np.exp(t) - 1.0) * w, rcond=None)[0]
    return [float(a[0] * scale), float(a[1] * scale ** 2), float(a[2] * scale ** 3)]


POLY_B = _fit_exp_poly()


def _register_dve_exp_op():
    """Define + register the custom DVE op (idempotent)."""
    from concourse.dve_spec import Spec, Src0, Src1, C0, C1, C2, One, lower
    from concourse.dve_ops import (
        DveOp, OPS, CUSTOM_DVE_SPECS, _SUB_OPCODE_FOR_NAME, _CUSTOM_DVE_ROW_BASE,
    )
    from concourse.dve_table_gen import dve_ver_for
    from concourse.dve_uop import DveOpSpec

    name = "EXP_POLY_MASK_ANT"
    if name in _SUB_OPCODE_FOR_NAME:
        return next(op for op in OPS if op.name == name)

    body = (((Src0 * C2 + C1) * Src0 + C0) * Src0 + One) * Src1
    spec = Spec(
        body=body,
        reference=lambda in0, in1, s0, s1, imm2: (
            (((in0 * imm2 + s1) * in0 + s0) * in0 + 1.0) * in1
        ),
    )
    op = DveOp(name, spec, subdim=False, uops_sha={})
    ver = dve_ver_for("TRN2")
    op.uops_sha[ver] = DveOpSpec(
        name=name, opcode=31, uops=lower(spec, ver=ver), rd1_en=True
    ).sha(ver)
    OPS.append(op)
    CUSTOM_DVE_SPECS[name] = spec
    _SUB_OPCODE_FOR_NAME[name] = _CUSTOM_DVE_ROW_BASE + len(OPS) - 1
    return op


# ---------------------------------------------------------------------------
# kernel build
# ---------------------------------------------------------------------------
def _build_kernel():
    import concourse.bacc as bacc
    import concourse.mybir as mybir
    from concourse.tile import TileContext
    from concourse.masks import make_identity

    EXP_OP = _register_dve_exp_op()

    F32 = mybir.dt.float32
    BF16 = mybir.dt.bfloat16
    FP8 = mybir.dt.float8e4
    AF = mybir.ActivationFunctionType
    ALU = mybir.AluOpType
    DR = mybir.MatmulPerfMode.DoubleRow

    nc = bacc.Bacc(None, target_bir_lowering=False, debug=False)

    def eng_copy(eng, dst, src):
        # NOTE: gpsimd cannot access PSUM on HW; keep psum reads on scalar/vector.
        if eng is nc.scalar:
            nc.scalar.copy(dst, src)
        else:
            eng.tensor_copy(dst, src)

    # ---- DRAM inputs (per core) ----
    xqtb = nc.dram_tensor("xqtb", [128, 2, NQC], BF16, kind="ExternalInput")
    xkt8 = nc.dram_tensor("xkt8", [128, 2, NK], FP8, kind="ExternalInput")
    wqb = nc.dram_tensor("wqb", [128, 2, H, A], BF16, kind="ExternalInput")
    wkTb = nc.dram_tensor("wkTb", [64, 2, H, 128], BF16, kind="ExternalInput")
    wv8 = nc.dram_tensor("wv8", [128, 2, H * A], FP8, kind="ExternalInput")
    wgtb = nc.dram_tensor("wgtb", [128, 2, H], BF16, kind="ExternalInput")
    bg = nc.dram_tensor("bg", [H, 1], F32, kind="ExternalInput")
    wo = nc.dram_tensor("wo", [A, DO], F32, kind="ExternalInput")
    bo = nc.dram_tensor("bo", [1, DO], F32, kind="ExternalInput")
    maskx = nc.dram_tensor("maskx", [NKC, 128, 2 * NQC], FP8, kind="ExternalInput")
    out = nc.dram_tensor("out", [NQC, DO], F32, kind="ExternalOutput")

    with TileContext(nc) as tc:
        with tc.sbuf_pool(name="consts", bufs=1) as cpool:
            # ---- constants ----
            wq_t = cpool.tile([128, 2, H, A], BF16)
            nc.sync.dma_start(wq_t, wqb[:])
            wkT_t = cpool.tile([64, 2, H, 128], BF16)
            nc.sync.dma_start(wkT_t, wkTb[:])
            wv_t = cpool.tile([128, 2, H * A], FP8)
            nc.sync.dma_start(wv_t, wv8[:])
            wgt_t = cpool.tile([128, 2, H], BF16)
            nc.sync.dma_start(wgt_t, wgtb[:])
            bg_t = cpool.tile([H, 1], F32)
            nc.sync.dma_start(bg_t, bg[:])
            xqtb_t = cpool.tile([128, 2, NQC], BF16)
            nc.sync.dma_start(xqtb_t, xqtb[:])
            xkt_t = cpool.tile([128, 2, NK], FP8)
            nc.sync.dma_start(xkt_t, xkt8[:])
            bo_t = cpool.tile([1, DO], F32)
            nc.sync.dma_start(bo_t, bo[:])
            wo_t = cpool.tile([A, DO], F32)
            nc.sync.dma_start(wo_t, wo[:])
            woaug = cpool.tile([A + 1, DO + 1], BF16)
            nc.vector.memset(woaug, 0.0)
            nc.any.tensor_copy(woaug[:A, :DO], wo_t)
            nc.vector.memset(woaug[A : A + 1, DO : DO + 1], 1.0)
            ones1 = cpool.tile([1, 128], F32)
            nc.vector.memset(ones1, 1.0)
            identity = cpool.tile([128, 128], F32)
            make_identity(nc, identity)

            # ---- persistent operand tiles ----
            # QW[h] = Wk_h @ Q_h^T in fp8 DR layout [128, 2, NQC] (c = i*128+p)
            qw8 = [cpool.tile([128, 2, NQC], FP8, name=f"qw{h}") for h in range(H)]
            qt_bf = cpool.tile([64, H, NQC], BF16)
            # last dim padded to 80 so the AV DoubleRow k-tile step is %16==0
            vaug = cpool.tile([128, H, NKC, 80], FP8)
            # only the augmented ones-column needs initialization
            nc.gpsimd.memset(vaug[:, :, :, A : A + 1], 1.0)
            gates = cpool.tile([H, NQC], F32)
            # whole mask image, SBUF resident (written in pass 1, reused pass 2)
            mask_sb = cpool.tile([128, NKC, 2 * NQC], FP8)
            nh = [cpool.tile([A + 1, NQC], BF16, name=f"nh{h}") for h in range(H)]

            KBLK = 512

            with (
                tc.psum_pool(name="pmain", bufs=1) as pm,
                tc.sbuf_pool(name="ms", bufs=1) as ms,
            ):
                # ---- build helpers (all ride the "sset" psum rotation) ----
                def sset_tile():
                    s4 = pm.tile([128, 2, NQC], F32, tag="sset", bufs=3,
                                 name="s4")
                    return s4

                def build_qt(hpair):
                    # Q_h^T = Wq_h^T @ x_Q^T  (bf16), heads 2*hpair, 2*hpair+1
                    qps = sset_tile()
                    for hh in range(2):
                        h = 2 * hpair + hh
                        for i in range(2):
                            nc.tensor.matmul(
                                qps[0:64, hh, :], wq_t[:, i, h, :],
                                xqtb_t[:, i, :],
                                start=(i == 0), stop=(i == 1),
                            )
                        eng_copy((nc.scalar, nc.vector)[hh], qt_bf[:, h, :],
                                 qps[0:64, hh, :])

                def build_qw(h):
                    # QW_h = Wk_h @ Q_h^T -> fp8 [128, 2, NQC] (c = i*128+p)
                    qps = sset_tile()
                    for half in range(2):
                        nc.tensor.matmul(
                            qps[:, half, :], wkT_t[:, half, h, :],
                            qt_bf[:, h, :],
                            start=True, stop=True,
                        )
                        eng_copy((nc.scalar, nc.vector)[half],
                                 qw8[h][:, half, :], qps[:, half, :])

                def build_v(P, cp):  # one key chunk PAIR, heads of pair P
                    vps = sset_tile()
                    for s in range(2):
                        c = 2 * cp + s
                        nc.tensor.matmul(
                            vps[:, s, 0 : 2 * A],
                            xkt_t[:, :, c * KC : (c + 1) * KC],
                            wv_t[:, :, 2 * P * A : (2 * P + 2) * A],
                            start=True, stop=True, perf_mode=DR,
                        )
                        eng_copy(
                            (nc.scalar, nc.scalar, nc.vector)[c % 3],
                            vaug[:, 2 * P : 2 * P + 2, c, 0:A],
                            vps[:, s, 0 : 2 * A].rearrange("p (h a) -> p h a", h=2),
                        )

                # ---- upfront mask DMAs (8 batched, sync engine) ----
                for g in range(8):
                    nc.sync.dma_start(
                        mask_sb[:, 8 * g : 8 * (g + 1), :],
                        maskx[8 * g : 8 * (g + 1)].rearrange("c p q -> p c q"),
                    )

                # ---- build: Q, QW (all heads), gates, V for pass 0 ----
                for hpair in range(2):
                    build_qt(hpair)
                for h in range(H):
                    build_qw(h)
                g_ps = sset_tile()
                for i in range(2):
                    nc.tensor.matmul(
                        g_ps[0:4, 0, :], wgt_t[:, i, :], xqtb_t[:, i, :],
                        start=(i == 0), stop=(i == 1),
                    )
                nc.scalar.activation(gates, g_ps[0:4, 0, :], AF.Sigmoid,
                                     bias=bg_t[:], scale=1.0)

                for cp in range(NPAIR):
                    build_v(0, cp)

                # ---- two passes over keys, one head pair each ----
                for PASS in range(2):
                    h0 = 2 * PASS
                    avP = [
                        pm.tile([A + 1, NQC], F32, tag=f"av{hh}", bufs=1,
                                name=f"av{hh}")
                        for hh in range(2)
                    ]
                    for pair in range(NPAIR):
                        em_cur = ms.tile([128, 2, 2, NQC], FP8, tag="em", bufs=3)

                        if PASS == 0 and pair < NPAIR // 2:
                            # interleave pass-1 V build into the pass-0 loop
                            build_v(1, 2 * pair)
                            build_v(1, 2 * pair + 1)

                        s4s = []
                        for slot in range(2):
                            c = 2 * pair + slot
                            s4 = sset_tile()
                            s4s.append(s4)
                            for hh in range(2):
                                nc.tensor.matmul(
                                    s4[:, hh, :],
                                    xkt_t[:, :, c * KC : (c + 1) * KC],
                                    qw8[h0 + hh],
                                    start=True, stop=True, perf_mode=DR,
                                )
                        for slot in range(2):
                            c = 2 * pair + slot
                            ct = CHUNK_TYPE[c]
                            dst = em_cur[:, slot]
                            if ct == "C":
                                nc.vector._custom_dve(
                                    EXP_OP, out=dst, in0=s4s[slot],
                                    in1=mask_sb[:, c, :],
                                    s0=POLY_B[0], s1=POLY_B[1], imm2=POLY_B[2],
                                )
                            else:
                                nc.scalar.activation(
                                    dst, s4s[slot], AF.Exp, bias=0.0, scale=0.125
                                )
                                dflat = dst.rearrange("p h q -> p (h q)")
                                nc.gpsimd.tensor_mul(dflat, dflat, mask_sb[:, c, :])
                        for hh in range(2):
                            nc.tensor.matmul(
                                avP[hh],
                                vaug[:, h0 + hh, 2 * pair : 2 * pair + 2, 0 : A + 1],
                                em_cur[:, :, hh, :],
                                start=(pair == 0), stop=(pair == NPAIR - 1),
                                perf_mode=DR,
                            )

                    for hh in range(2):
                        eng_copy((nc.scalar, nc.vector)[hh], nh[h0 + hh], avP[hh])

            # ---------------- epilogue ----------------
            with (
                tc.psum_pool(name="pe", bufs=1) as pm,
                tc.sbuf_pool(name="es", bufs=1) as ms,
            ):
                gt_ps = pm.tile([128, 4 * H], F32, tag="gt", bufs=1)
                for qtile in range(4):
                    nc.tensor.transpose(
                        gt_ps[:, qtile * H : qtile * H + H],
                        gates[:, qtile * 128 : (qtile + 1) * 128],
                        identity[:H, :H],
                    )
                gt_sb = ms.tile([128, 4 * H], F32, tag="gtsb", bufs=1)
                nc.any.tensor_copy(gt_sb, gt_ps)
                boB_ps = pm.tile([128, DO], F32, tag="bob", bufs=1)
                nc.tensor.matmul(boB_ps, ones1, bo_t, start=True, stop=True)
                boB = ms.tile([128, DO], F32, tag="bobsb", bufs=1)
                nc.any.tensor_copy(boB, boB_ps)
                for qtile in range(4):
                    acc = boB
                    for h in range(H):
                        p_ps = pm.tile([128, DO + 1], F32, tag="p", bufs=2)
                        nc.tensor.matmul(
                            p_ps,
                            nh[h][:, qtile * 128 : (qtile + 1) * 128],
                            woaug,
                            start=True, stop=True,
                        )
                        rden = ms.tile([128, 1], F32, tag="rden", bufs=2)
                        nc.vector.reciprocal(rden, p_ps[:, DO : DO + 1])
                        sc = ms.tile([128, 1], F32, tag="sc", bufs=2)
                        nc.any.tensor_mul(
                            sc, rden, gt_sb[:, qtile * H + h : qtile * H + h + 1]
                        )
                        nxt = ms.tile([128, DO], F32, tag=f"acc{h % 2}", bufs=2)
                        nc.vector.scalar_tensor_tensor(
                            nxt, p_ps[:, :DO], sc, acc,
                            op0=ALU.mult, op1=ALU.add,
                        )
                        acc = nxt
                    nc.sync.dma_start(
                        out[qtile * 128 : (qtile + 1) * 128, :], acc
                    )
    nc.finalize()
    return nc


# ---------------------------------------------------------------------------
# host-side input prep
# ---------------------------------------------------------------------------
def _to_f8(x):
    import ml_dtypes
    return np.ascontiguousarray(np.asarray(x, dtype=np.float32).astype(
        ml_dtypes.float8_e4m3fn))


def _to_bf16(x):
    import ml_dtypes
    return np.ascontiguousarray(np.asarray(x, dtype=np.float32).astype(
        ml_dtypes.bfloat16))


def _dr_c_layout(xT):
    """[C=256, N] -> [128, 2, N] with c = i*128 + p."""
    return np.ascontiguousarray(xT.reshape(2, 128, -1).transpose(1, 0, 2))


def _prep_shared(x_K, Wq, Wk, Wv, Wg, bg, Wo, bo):
    xkt = x_K.T  # [256, NK]
    xkt8 = _to_f8(_dr_c_layout(xkt))

    # wqb[p, i, h, a] = Wq[h, i*128+p, a]
    wqb = _to_bf16(Wq.transpose(1, 0, 2).reshape(2, 128, H, A).transpose(1, 0, 2, 3))
    # wkTb[a, half, h, m] = Wk[h, 128*half + m, a]
    wkTb = _to_bf16(
        Wk.reshape(H, 2, 128, A).transpose(3, 1, 0, 2)
    )
    arr = np.empty((128, 2, H * A), np.float32)
    for h in range(H):
        arr[:, :, h * A:(h + 1) * A] = Wv[h].reshape(2, 128, A).transpose(1, 0, 2)
    wv8 = _to_f8(arr)
    wgtb = _to_bf16(Wg.T.reshape(2, 128, H).transpose(1, 0, 2))
    return {
        "xkt8": xkt8, "wqb": wqb, "wkTb": wkTb, "wv8": wv8, "wgtb": wgtb,
        "bg": np.asarray(bg, np.float32).reshape(H, 1),
        "wo": np.ascontiguousarray(np.asarray(Wo, np.float32)),
        "bo": np.asarray(bo, np.float32).reshape(1, DO),
    }


def _prep_mask_core(mask_sl):
    """mask_sl: [NQC, NK] int32 -> maskx [NKC, 128, 2*NQC] fp8 (0/1, duplicated
    per head so [p, (h q)] reads align with the em layout)."""
    import ml_dtypes
    mt = mask_sl.T.astype(np.float32)  # [NK, NQC]
    m3 = mt.reshape(NKC, KC, NQC)
    maskx = np.concatenate([m3, m3], axis=2)  # [NKC, 128, 2*NQC]
    return np.ascontiguousarray(maskx.astype(ml_dtypes.float8_e4m3fn))


def kernel(x_Q, x_K, mask, Wq, Wk, Wv, Wg, bg, Wo, bo):
    from concourse.bass_utils import run_bass_kernel_spmd

    x_Q = np.asarray(x_Q, dtype=np.float32)
    x_K = np.asarray(x_K, dtype=np.float32)
    mask = np.asarray(mask, dtype=np.int32)

    shared = _prep_shared(
        x_K, np.asarray(Wq, np.float32), np.asarray(Wk, np.float32),
        np.asarray(Wv, np.float32), np.asarray(Wg, np.float32),
        bg, Wo, bo,
    )

    in_maps = []
    for cidx in range(NCORES):
        sl = slice(cidx * NQC, (cidx + 1) * NQC)
        xqt = x_Q[sl].T  # [256, NQC]
        m = {
            "xqtb": _to_bf16(_dr_c_layout(xqt)),
            "maskx": _prep_mask_core(mask[sl]),
        }
        m.update(shared)
        in_maps.append(m)

    if "nc" not in _cache:
        _cache["nc"] = _build_kernel()
    res = run_bass_kernel_spmd(
        _cache["nc"], in_maps, list(range(NCORES)),
        trace=bool(int(os.environ.get("BASS_KERNEL_TRACE", "0"))),
    )
    if res.exec_time_ns is not None:
        print(f"HW exec time: {res.exec_time_ns} ns")
    return np.concatenate([r["out"] for r in res.results], axis=0)


# revision 20
# speedup vs baseline: 1.2386x; 1.2188x over previous
"""Trainium2 Bass kernel for KeyValueAttention (4-head masked attention, gated combine).

v3 strategy (8 NeuronCores, query-dim sharded, 512 queries/core):
  Transposed space throughout (keys/features on partitions, queries on free dim).
  - All projections (Q/K/V) are fp8e4 DoubleRow matmuls (contraction 256 as
    2x128 k-tiles) -> 0.5 cycles/row on the PE.
  - TWO PASSES over the keys, one per head pair. Per pass the scores psum
    rotates through 3 buffers (6 banks) and the 2 AV accumulators use 2 banks,
    fitting the 8-bank PSUM while keeping the exp pipeline deep.
  - Scores: fp8 DR matmul, contraction A=64 as 2x32 k-tiles:
    lhsT = K^T chunk [32, 2, 128], rhs = Q^T [32, 2, 512] -> psum [128k, 512q].
  - Masked exp alternates engines by chunk parity:
      * even chunks (ACT): mask pre-added as -160 bias via an identity DR
        matmul opening the psum accumulation group, then ACT Exp (scale=1/8).
      * odd chunks (DVE): custom DVE op computes cubic-poly exp(s/8) * mask
        stream in one pass (Src0 = psum scores, Src1 = fp8 mask from SBUF).
    Both write em directly as fp8e4.
  - The fp8 mask image for all chunks is DMA'd into SBUF once (pass 1) and
    reused from SBUF in pass 2.
  - AV: fp8 DR over chunk pairs: lhsT = Vaug [128, 2, 65], rhs = em
    [128, 2, 512] -> psum [65, 512] per head; row 64 = softmax denominator.
  - The pass-2 K/V build matmuls are interleaved into the pass-1 chunk loop.

Host side only reshapes/slices/transposes/casts inputs (no reference math).
"""

import os
import numpy as np

NQ, NK, DC, A, H, DO = 4096, 8192, 256, 64, 4, 256
NCORES = 8
NQC = NQ // NCORES   # 512 queries per core
KC = 128             # keys per chunk
NKC = NK // KC       # 64 chunks
NPAIR = NKC // 2     # 32 chunk pairs

# chunk -> exp/mask path:
#   'C'  = DVE custom op (poly exp * mask stream), one pass
#   'Bd' = ACT exp (unmasked) + DVE in-place mask multiply
#   'Bp' = ACT exp (unmasked) + gpsimd in-place mask multiply
def _slot_type(s):
    # ~40% of chunk-slots pair ACT exp with a gpsimd mask-multiply; the rest
    # run the fused poly+mask on the DVE.
    return "Bp" if s % 5 < 2 else "C"

SLOT_TYPE = [_slot_type(s) for s in range(4 * NKC)]

_cache = {}


# ---------------------------------------------------------------------------
# exp polynomial fit (shared host/device constants)
# ---------------------------------------------------------------------------
def _fit_exp_poly(scale=0.125, lo=-0.85, hi=0.85):
    """p(x) = 1 + b1 x + b2 x^2 + b3 x^3 ~ exp(x*scale) for x*scale in [lo,hi],
    relative-error weighted, p(0)=1 pinned."""
    t = np.linspace(lo, hi, 40001)
    w = 1.0 / np.exp(t)
    Amat = np.stack([t, t * t, t ** 3], axis=1) * w[:, None]
    a = np.linalg.lstsq(Amat, (np.exp(t) - 1.0) * w, rcond=None)[0]
    return [float(a[0] * scale), float(a[1] * scale ** 2), float(a[2] * scale ** 3)]


POLY_B = _fit_exp_poly()


def _register_dve_exp_op():
    """Define + register the custom DVE op (idempotent)."""
    from concourse.dve_spec import Spec, Src0, Src1, C0, C1, C2, One, lower
    from concourse.dve_ops import (
        DveOp, OPS, CUSTOM_DVE_SPECS, _SUB_OPCODE_FOR_NAME, _CUSTOM_DVE_ROW_BASE,
    )
    from concourse.dve_table_gen import dve_ver_for
    from concourse.dve_uop import DveOpSpec

    name = "EXP_POLY_MASK_ANT"
    if name in _SUB_OPCODE_FOR_NAME:
        return next(op for op in OPS if op.name == name)

    body = (((Src0 * C2 + C1) * Src0 + C0) * Src0 + One) * Src1
    spec = Spec(
        body=body,
        reference=lambda in0, in1, s0, s1, imm2: (
            (((in0 * imm2 + s1) * in0 + s0) * in0 + 1.0) * in1
        ),
    )
    op = DveOp(name, spec, subdim=False, uops_sha={})
    ver = dve_ver_for("TRN2")
    op.uops_sha[ver] = DveOpSpec(
        name=name, opcode=31, uops=lower(spec, ver=ver), rd1_en=True
    ).sha(ver)
    OPS.append(op)
    CUSTOM_DVE_SPECS[name] = spec
    _SUB_OPCODE_FOR_NAME[name] = _CUSTOM_DVE_ROW_BASE + len(OPS) - 1
    return op


# ---------------------------------------------------------------------------
# kernel build
# ---------------------------------------------------------------------------
def _build_kernel():
    import concourse.bacc as bacc
    import concourse.mybir as mybir
    from concourse.tile import TileContext
    from concourse.masks import make_identity

    EXP_OP = _register_dve_exp_op()

    F32 = mybir.dt.float32
    BF16 = mybir.dt.bfloat16
    FP8 = mybir.dt.float8e4
    AF = mybir.ActivationFunctionType
    ALU = mybir.AluOpType
    DR = mybir.MatmulPerfMode.DoubleRow

    nc = bacc.Bacc(None, target_bir_lowering=False, debug=False)

    def eng_copy(eng, dst, src):
        # NOTE: gpsimd cannot access PSUM on HW; keep psum reads on scalar/vector.
        if eng is nc.scalar:
            nc.scalar.copy(dst, src)
        else:
            eng.tensor_copy(dst, src)

    # ---- DRAM inputs (per core) ----
    xqtb = nc.dram_tensor("xqtb", [128, 2, NQC], BF16, kind="ExternalInput")
    xkt8 = nc.dram_tensor("xkt8", [128, 2, NK], FP8, kind="ExternalInput")
    wqb = nc.dram_tensor("wqb", [128, 2, H, A], BF16, kind="ExternalInput")
    wkTb = nc.dram_tensor("wkTb", [64, 2, H, 128], BF16, kind="ExternalInput")
    wv8 = nc.dram_tensor("wv8", [128, 2, H * A], FP8, kind="ExternalInput")
    wgtb = nc.dram_tensor("wgtb", [128, 2, H], BF16, kind="ExternalInput")
    bg = nc.dram_tensor("bg", [H, 1], F32, kind="ExternalInput")
    wo = nc.dram_tensor("wo", [A, DO], F32, kind="ExternalInput")
    bo = nc.dram_tensor("bo", [1, DO], F32, kind="ExternalInput")
    maskx = nc.dram_tensor("maskx", [NKC, 128, NQC], FP8, kind="ExternalInput")
    out = nc.dram_tensor("out", [NQC, DO], F32, kind="ExternalOutput")

    with TileContext(nc) as tc:
        with tc.sbuf_pool(name="consts", bufs=1) as cpool:
            # ---- constants ----
            wq_t = cpool.tile([128, 2, H, A], BF16)
            nc.sync.dma_start(wq_t, wqb[:])
            wkT_t = cpool.tile([64, 2, H, 128], BF16)
            nc.sync.dma_start(wkT_t, wkTb[:])
            wv_t = cpool.tile([128, 2, H * A], FP8)
            nc.sync.dma_start(wv_t, wv8[:])
            wgt_t = cpool.tile([128, 2, H], BF16)
            nc.sync.dma_start(wgt_t, wgtb[:])
            bg_t = cpool.tile([H, 1], F32)
            nc.sync.dma_start(bg_t, bg[:])
            xqtb_t = cpool.tile([128, 2, NQC], BF16)
            nc.sync.dma_start(xqtb_t, xqtb[:])
            xkt_t = cpool.tile([128, 2, NK], FP8)
            nc.sync.dma_start(xkt_t, xkt8[:])
            bo_t = cpool.tile([1, DO], F32)
            nc.sync.dma_start(bo_t, bo[:])
            wo_t = cpool.tile([A, DO], F32)
            nc.sync.dma_start(wo_t, wo[:])
            woaug = cpool.tile([A + 1, DO + 1], BF16)
            nc.vector.memset(woaug, 0.0)
            nc.any.tensor_copy(woaug[:A, :DO], wo_t)
            nc.vector.memset(woaug[A : A + 1, DO : DO + 1], 1.0)
            ones1 = cpool.tile([1, 128], F32)
            nc.vector.memset(ones1, 1.0)
            identity = cpool.tile([128, 128], F32)
            make_identity(nc, identity)

            # ---- persistent operand tiles ----
            # QW[h] = Wk_h @ Q_h^T in fp8 DR layout [128, 2, NQC] (c = i*128+p)
            qw8 = [cpool.tile([128, 2, NQC], FP8, name=f"qw{h}") for h in range(H)]
            qt_bf = cpool.tile([64, H, NQC], BF16)
            # last dim padded to 80 so the AV DoubleRow k-tile step is %16==0
            vaug = cpool.tile([128, H, NKC, 80], FP8)
            # only the augmented ones-column needs initialization
            nc.gpsimd.memset(vaug[:, :, :, A : A + 1], 1.0)
            gates = cpool.tile([H, NQC], F32)
            # whole mask image, SBUF resident (loaded once, reused by all passes)
            mask_sb = cpool.tile([128, NKC, NQC], FP8)
            nh = [cpool.tile([A + 1, NQC], BF16, name=f"nh{h}") for h in range(H)]

            KBLK = 512

            with (
                tc.psum_pool(name="pmain", bufs=1) as pm,
                tc.sbuf_pool(name="ms", bufs=1) as ms,
            ):
                # ---- build helpers (all ride the "sset" psum rotation) ----
                def sset_tile():
                    s4 = pm.tile([128, NQC], F32, tag="sset", bufs=7,
                                 name="s4")
                    return s4

                def build_qt(h):
                    # Q_h^T = Wq_h^T @ x_Q^T  (bf16)
                    qps = sset_tile()
                    for i in range(2):
                        nc.tensor.matmul(
                            qps[0:64, :], wq_t[:, i, h, :], xqtb_t[:, i, :],
                            start=(i == 0), stop=(i == 1),
                        )
                    eng_copy((nc.scalar, nc.vector)[h % 2], qt_bf[:, h, :],
                             qps[0:64, :])

                def build_qw(h):
                    # QW_h = Wk_h @ Q_h^T -> fp8 [128, 2, NQC] (c = i*128+p)
                    for half in range(2):
                        qps = sset_tile()
                        nc.tensor.matmul(
                            qps, wkT_t[:, half, h, :], qt_bf[:, h, :],
                            start=True, stop=True,
                        )
                        eng_copy((nc.scalar, nc.vector)[half],
                                 qw8[h][:, half, :], qps)

                def build_v(P, cp):  # one key chunk PAIR, heads of pair P
                    vps = sset_tile()
                    for s in range(2):
                        c = 2 * cp + s
                        nc.tensor.matmul(
                            vps[:, s * 2 * A : (s + 1) * 2 * A],
                            xkt_t[:, :, c * KC : (c + 1) * KC],
                            wv_t[:, :, 2 * P * A : (2 * P + 2) * A],
                            start=True, stop=True, perf_mode=DR,
                        )
                        eng_copy(
                            (nc.scalar, nc.scalar, nc.vector)[c % 3],
                            vaug[:, 2 * P : 2 * P + 2, c, 0:A],
                            vps[:, s * 2 * A : (s + 1) * 2 * A].rearrange(
                                "p (h a) -> p h a", h=2),
                        )

                # ---- upfront mask DMAs (8 batched, sync engine) ----
                for g in range(8):
                    nc.sync.dma_start(
                        mask_sb[:, 8 * g : 8 * (g + 1), :],
                        maskx[8 * g : 8 * (g + 1)].rearrange("c p q -> p c q"),
                    )

                # ---- build: Q, QW (all heads), gates, V for passes 0/1 ----
                for h in range(H):
                    build_qt(h)
                    build_qw(h)
                g_ps = sset_tile()
                for i in range(2):
                    nc.tensor.matmul(
                        g_ps[0:4, :], wgt_t[:, i, :], xqtb_t[:, i, :],
                        start=(i == 0), stop=(i == 1),
                    )
                nc.scalar.activation(gates, g_ps[0:4, :], AF.Sigmoid,
                                     bias=bg_t[:], scale=1.0)

                for cp in range(NPAIR):
                    build_v(0, cp)

                # ---- four passes over keys, one head each ----
                for h in range(H):
                    avh = pm.tile([A + 1, NQC], F32, tag="av", bufs=1,
                                  name="avh")
                    for pair in range(NPAIR):
                        em_cur = ms.tile([128, 2, NQC], FP8, tag="em", bufs=4)

                        if h == 0 and pair < NPAIR // 2:
                            # interleave the heads-2/3 V build into pass 0
                            build_v(1, 2 * pair)
                            build_v(1, 2 * pair + 1)

                        s4s = []
                        for slot in range(2):
                            c = 2 * pair + slot
                            s4 = sset_tile()
                            s4s.append(s4)
                            nc.tensor.matmul(
                                s4,
                                xkt_t[:, :, c * KC : (c + 1) * KC],
                                qw8[h],
                                start=True, stop=True, perf_mode=DR,
                            )
                        for slot in range(2):
                            c = 2 * pair + slot
                            ct = SLOT_TYPE[h * NKC + c]
                            dst = em_cur[:, slot]
                            if ct == "C":
                                nc.vector._custom_dve(
                                    EXP_OP, out=dst, in0=s4s[slot],
                                    in1=mask_sb[:, c, :],
                                    s0=POLY_B[0], s1=POLY_B[1], imm2=POLY_B[2],
                                )
                            else:
                                nc.scalar.activation(
                                    dst, s4s[slot], AF.Exp, bias=0.0, scale=0.125
                                )
                                nc.gpsimd.tensor_mul(dst, dst, mask_sb[:, c, :])
                        nc.tensor.matmul(
                            avh,
                            vaug[:, h, 2 * pair : 2 * pair + 2, 0 : A + 1],
                            em_cur,
                            start=(pair == 0), stop=(pair == NPAIR - 1),
                            perf_mode=DR,
                        )

                    eng_copy((nc.scalar, nc.vector)[h % 2], nh[h], avh)

            # ---------------- epilogue ----------------
            with (
                tc.psum_pool(name="pe", bufs=1) as pm,
                tc.sbuf_pool(name="es", bufs=1) as ms,
            ):
                gt_ps = pm.tile([128, 4 * H], F32, tag="gt", bufs=1)
                for qtile in range(4):
                    nc.tensor.transpose(
                        gt_ps[:, qtile * H : qtile * H + H],
                        gates[:, qtile * 128 : (qtile + 1) * 128],
                        identity[:H, :H],
                    )
                gt_sb = ms.tile([128, 4 * H], F32, tag="gtsb", bufs=1)
                nc.any.tensor_copy(gt_sb, gt_ps)
                boB_ps = pm.tile([128, DO], F32, tag="bob", bufs=1)
                nc.tensor.matmul(boB_ps, ones1, bo_t, start=True, stop=True)
                boB = ms.tile([128, DO], F32, tag="bobsb", bufs=1)
                nc.any.tensor_copy(boB, boB_ps)
                for qtile in range(4):
                    acc = boB
                    for h in range(H):
                        p_ps = pm.tile([128, DO + 1], F32, tag="p", bufs=2)
                        nc.tensor.matmul(
                            p_ps,
                            nh[h][:, qtile * 128 : (qtile + 1) * 128],
                            woaug,
                            start=True, stop=True,
                        )
                        rden = ms.tile([128, 1], F32, tag="rden", bufs=2)
                        nc.vector.reciprocal(rden, p_ps[:, DO : DO + 1])
                        sc = ms.tile([128, 1], F32, tag="sc", bufs=2)
                        nc.any.tensor_mul(
                            sc, rden, gt_sb[:, qtile * H + h : qtile * H + h + 1]
                        )
                        nxt = ms.tile([128, DO], F32, tag=f"acc{h % 2}", bufs=2)
                        nc.vector.scalar_tensor_tensor(
                            nxt, p_ps[:, :DO], sc, acc,
                            op0=ALU.mult, op1=ALU.add,
                        )
                        acc = nxt
                    nc.sync.dma_start(
                        out[qtile * 128 : (qtile + 1) * 128, :], acc
                    )
    nc.finalize()
    return nc


# ---------------------------------------------------------------------------
# host-side input prep
# ---------------------------------------------------------------------------
def _to_f8(x):
    import ml_dtypes
    return np.ascontiguousarray(np.asarray(x, dtype=np.float32).astype(
        ml_dtypes.float8_e4m3fn))


def _to_bf16(x):
    import ml_dtypes
    return np.ascontiguousarray(np.asarray(x, dtype=np.float32).astype(
        ml_dtypes.bfloat16))


def _dr_c_layout(xT):
    """[C=256, N] -> [128, 2, N] with c = i*128 + p."""
    return np.ascontiguousarray(xT.reshape(2, 128, -1).transpose(1, 0, 2))


def _prep_shared(x_K, Wq, Wk, Wv, Wg, bg, Wo, bo):
    xkt = x_K.T  # [256, NK]
    xkt8 = _to_f8(_dr_c_layout(xkt))

    # wqb[p, i, h, a] = Wq[h, i*128+p, a]
    wqb = _to_bf16(Wq.transpose(1, 0, 2).reshape(2, 128, H, A).transpose(1, 0, 2, 3))
    # wkTb[a, half, h, m] = Wk[h, 128*half + m, a]
    wkTb = _to_bf16(
        Wk.reshape(H, 2, 128, A).transpose(3, 1, 0, 2)
    )
    arr = np.empty((128, 2, H * A), np.float32)
    for h in range(H):
        arr[:, :, h * A:(h + 1) * A] = Wv[h].reshape(2, 128, A).transpose(1, 0, 2)
    wv8 = _to_f8(arr)
    wgtb = _to_bf16(Wg.T.reshape(2, 128, H).transpose(1, 0, 2))
    return {
        "xkt8": xkt8, "wqb": wqb, "wkTb": wkTb, "wv8": wv8, "wgtb": wgtb,
        "bg": np.asarray(bg, np.float32).reshape(H, 1),
        "wo": np.ascontiguousarray(np.asarray(Wo, np.float32)),
        "bo": np.asarray(bo, np.float32).reshape(1, DO),
    }


def _prep_mask_core(mask_sl):
    """mask_sl: [NQC, NK] int32 -> maskx [NKC, 128, NQC] fp8 0/1."""
    import ml_dtypes
    mt = mask_sl.T.astype(np.float32)  # [NK, NQC]
    m3 = mt.reshape(NKC, KC, NQC)
    return np.ascontiguousarray(m3.astype(ml_dtypes.float8_e4m3fn))


def kernel(x_Q, x_K, mask, Wq, Wk, Wv, Wg, bg, Wo, bo):
    from concourse.bass_utils import run_bass_kernel_spmd

    x_Q = np.asarray(x_Q, dtype=np.float32)
    x_K = np.asarray(x_K, dtype=np.float32)
    mask = np.asarray(mask, dtype=np.int32)

    shared = _prep_shared(
        x_K, np.asarray(Wq, np.float32), np.asarray(Wk, np.float32),
        np.asarray(Wv, np.float32), np.asarray(Wg, np.float32),
        bg, Wo, bo,
    )

    in_maps = []
    for cidx in range(NCORES):
        sl = slice(cidx * NQC, (cidx + 1) * NQC)
        xqt = x_Q[sl].T  # [256, NQC]
        m = {
            "xqtb": _to_bf16(_dr_c_layout(xqt)),
            "maskx": _prep_mask_core(mask[sl]),
        }
        m.update(shared)
        in_maps.append(m)

    if "nc" not in _cache:
        _cache["nc"] = _build_kernel()
    res = run_bass_kernel_spmd(
        _cache["nc"], in_maps, list(range(NCORES)),
        trace=bool(int(os.environ.get("BASS_KERNEL_TRACE", "0"))),
    )
    if res.exec_time_ns is not None:
        print(f"HW exec time: {res.exec_time_ns} ns")
    return np.concatenate([r["out"] for r in res.results], axis=0)
